# revision 27
# baseline (speedup 1.0000x reference)
"""Trainium2 Bass kernel for nn_MixedGatedMLP (4-bit quantized gated MLP + LoRA).

Strategy v2: tensor-parallel over d_ff across 8 NeuronCores (F padded
11008->11264, FS=1408 rows/core).  Host ships unpacked nibble planes (u8) and
compact per-64-block absmax rows; the device does the codebook lookup
(16x is_equal/mult tensor_scalar at 4x DVE rate + an add tree partially
offloaded to GPSIMD) and blockwise scaling (scale plane expanded on PE via a
2->128 one-hot matmul, multiplied straight out of PSUM).

Phase 1 streams x in token tiles over ramped f-slices of the gate/up weights;
dequant of slice s+1 overlaps the matmuls of slice s (issue-order zip).
SiLU runs on ACT, the gating multiply on GPSIMD, and x3 goes to DRAM.
The down weights dequant during the last slices' matmuls and round-trip
through DRAM.  Phase 2 recomputes y3 = x3 @ wd + lora in d-halves, with a
token-quartered bf16 ReduceScatter overlapping the next quarter's matmuls.
Core i returns tokens {TQ*q + TQC*i ..} for each quarter q; the host
reassembles.
"""

import sys

for _p in ("/opt/trn_rl_repo", "/root/.axon_site/_ro/trn_rl_repo"):
    if _p not in sys.path:
        sys.path.append(_p)

from contextlib import ExitStack

import numpy as np
import ml_dtypes

import concourse.bass as bass
import concourse.mybir as mybir
import concourse.tile as tile
from concourse import bacc
from concourse.bass_utils import run_bass_kernel_spmd

BF16 = ml_dtypes.bfloat16
ALU = mybir.AluOpType
AFT = mybir.ActivationFunctionType


class Cfg:
    def __init__(self, D=4096, T=4096, F=11008, R=16, block=64, ncores=8,
                 use_silu=True):
        self.D = D
        self.T = T
        self.F = F
        self.R = R
        self.block = block
        self.ncores = ncores
        self.use_silu = use_silu

        unit = 128 * ncores
        self.FP = ((F + unit - 1) // unit) * unit   # padded d_ff
        self.FS = self.FP // ncores                 # per-core f rows
        self.NFG = self.FS // 128                   # 128-row f groups
        self.DCH = D // 128                         # 128-row d chunks
        self.NAB = D // block                       # absmax rows (gate/up)
        self.ABC = 128 // block                     # absmax blocks per chunk

        # token tiling
        self.TTW = min(512, T)                      # phase-1 token tile width
        self.NT = T // self.TTW
        self.NTG = T // 128                         # phase-2 token groups

        # phase-1 f slices in fgroup units: ramped for a small first bubble
        if self.NFG == 11:
            self.slices = [1, 2, 3, 2, 3]
        else:
            self.slices = [1] * self.NFG

        self.gp_terms = 6        # codebook terms handled by gpsimd (g/u)
        self.gp_terms_down = 8   # ... for the down weights (gpsimd idle then)
        self.deq_w = 2048        # target dequant op width (free elems)

        # phase 2
        self.n_dh = 2 if D >= 2048 else 1           # d halves
        self.DH = D // self.n_dh
        self.n_q = 4 if T >= 4096 else 2            # token quarters (RS chunks)
        self.TQ = T // self.n_q
        self.TQC = self.TQ // ncores                # rows per core per quarter
        self.TS = T // ncores

    def slice_groups(self, nfg):
        """Dequant chunk-groups (c0, G) for a slice of nfg fgroups."""
        fw = nfg * 128
        G = max(1, min(self.deq_w // fw, 8, self.DCH))
        while self.DCH % G:
            G -= 1
        return [(c0, G) for c0 in range(0, self.DCH, G)]


def build_graph(cfg: Cfg):
    nc = bacc.Bacc(None, num_devices=cfg.ncores)
    dt = mybir.dt
    D, T, FS, R, NFG = cfg.D, cfg.T, cfg.FS, cfg.R, cfg.NFG
    TTW = cfg.TTW
    rg = [list(range(cfg.ncores))]

    # ---- external inputs (per-core) ----
    xT = nc.dram_tensor("xT", [D, T], dt.bfloat16, kind="ExternalInput")
    g_nib = nc.dram_tensor("g_nib", [D, FS], dt.uint8, kind="ExternalInput")
    u_nib = nc.dram_tensor("u_nib", [D, FS], dt.uint8, kind="ExternalInput")
    d_nib = nc.dram_tensor("d_nib", [FS, D], dt.uint8, kind="ExternalInput")
    g_amp = nc.dram_tensor("g_amp", [D, FS], dt.bfloat16, kind="ExternalInput")
    u_amp = nc.dram_tensor("u_amp", [D, FS], dt.bfloat16, kind="ExternalInput")
    d_amp = nc.dram_tensor("d_amp", [FS, D], dt.bfloat16, kind="ExternalInput")
    code_rep = nc.dram_tensor("code_rep", [128, 16], dt.float32, kind="ExternalInput")
    a_gu = nc.dram_tensor("a_gu", [D, 2 * R], dt.bfloat16, kind="ExternalInput")
    b_g = nc.dram_tensor("b_g", [R, FS], dt.bfloat16, kind="ExternalInput")
    b_u = nc.dram_tensor("b_u", [R, FS], dt.bfloat16, kind="ExternalInput")
    a_d = nc.dram_tensor("a_d", [FS, R], dt.bfloat16, kind="ExternalInput")
    b_d = nc.dram_tensor("b_d", [R, D], dt.bfloat16, kind="ExternalInput")

    y_out = nc.dram_tensor("y_out", [cfg.TS, D], dt.float32, kind="ExternalOutput")

    # ---- internal DRAM ----
    x3_dram = nc.dram_tensor("x3_dram", [FS, T], dt.bfloat16, kind="Internal")
    wd_dram = nc.dram_tensor("wd_dram", [FS, D], dt.bfloat16, kind="Internal")
    rs_in = [
        nc.dram_tensor(f"rs_in{i}", [cfg.TQ, cfg.DH], dt.bfloat16, kind="Internal")
        for i in range(2)
    ]
    rs_out = [
        nc.dram_tensor(f"rs_out{i}", [cfg.TQC, cfg.DH], dt.bfloat16,
                       kind="Internal")
        for i in range(2)
    ]

    with tile.TileContext(nc) as tc, ExitStack() as ctx:
        # ---------------- constants ----------------
        cpool = ctx.enter_context(tc.tile_pool(name="const", bufs=1))
        code_sb = cpool.tile([128, 16], dt.float32)
        nc.sync.dma_start(code_sb[:], code_rep[:])
        agu_sb = cpool.tile([128, cfg.DCH, 2 * R], dt.bfloat16)
        nc.sync.dma_start(agu_sb[:], a_gu.rearrange("(c p) r -> p c r", p=128))
        bg_sb = cpool.tile([R, FS], dt.bfloat16)
        nc.sync.dma_start(bg_sb[:], b_g[:])
        bu_sb = cpool.tile([R, FS], dt.bfloat16)
        nc.sync.dma_start(bu_sb[:], b_u[:])
        ad_sb = cpool.tile([128, NFG, R], dt.bfloat16)
        nc.sync.dma_start(ad_sb[:], a_d.rearrange("(c p) r -> p c r", p=128))
        xag_sb = cpool.tile([R, T], dt.bfloat16)
        xau_sb = cpool.tile([R, T], dt.bfloat16)
        x3a_sb = cpool.tile([R, T], dt.bfloat16)

        dqp_cm = tc.tile_pool(name="dq", bufs=1)
        dqp = dqp_cm.__enter__()

        # ------------- phase 1 + dequant -------------
        wtiles = {}          # si -> (wg, wu); even/odd parity tags share bufs
        tasks = []

        def pop_tasks(n):
            for _ in range(n):
                if tasks:
                    tasks.pop(0)()

        with tc.tile_pool(name="w", bufs=1) as wp, \
             tc.tile_pool(name="xt", bufs=1) as xtp, \
             tc.tile_pool(name="p1", bufs=1) as p1p, \
             tc.tile_pool(name="ps1", bufs=1, space="PSUM") as psp:

            def open_wpool(si):
                if si in wtiles:
                    return
                fw = cfg.slices[si] * 128
                par = "eo"[si % 2]
                wg = wp.tile([128, cfg.DCH, fw], dt.bfloat16, tag=f"wg{par}",
                             name=f"wg{si}")
                wu = wp.tile([128, cfg.DCH, fw], dt.bfloat16, tag=f"wu{par}",
                             name=f"wu{si}")
                wtiles[si] = (wg, wu)

            def emit_deq(nib, amp, c0, G, f0, fw, wview, gp, out_dram=None):
                """Dequant chunk rows [128*c0, 128*(c0+G)) x cols [f0, f0+fw)
                into bf16 weights at `wview` ([128, G, fw] AP); optionally DMA
                to out_dram."""
                GW = G * fw
                NB = dqp.tile([128, G, fw], dt.uint8, tag="nb", bufs=2,
                              name="nb")
                nc.sync.dma_start(
                    NB[:], nib[128 * c0:128 * (c0 + G), f0:f0 + fw]
                    .rearrange("(g p) f -> p g f", p=128))
                S = dqp.tile([128, G, fw], dt.bfloat16, tag="am", bufs=2,
                             name="S")
                nc.sync.dma_start(
                    S[:], amp[128 * c0:128 * (c0 + G), f0:f0 + fw]
                    .rearrange("(g p) f -> p g f", p=128))
                X = dqp.tile([128, G, fw], dt.bfloat16, tag="x", bufs=1,
                             name="x")
                nc.vector.tensor_scalar(X[:], NB[:], 1.0, None, ALU.mult)
                Xf = X[:].rearrange("p g f -> p (g f)")
                acc0 = dqp.tile([128, GW], dt.bfloat16, tag="a0", name="a0")
                acc1 = dqp.tile([128, GW], dt.bfloat16, tag="a1", name="a1")
                if GW < 1024:
                    gp = 0
                accg = None
                if gp:
                    accg = dqp.tile([128, GW], dt.bfloat16, tag="ag",
                                    name="ag")
                nterm = 0
                for k in range(16):
                    sc2 = code_sb[:, k:k + 1]
                    if k == 0:
                        nc.vector.tensor_scalar(acc0[:], Xf, 0.0, sc2,
                                                ALU.is_equal, ALU.mult)
                    elif k == 1:
                        nc.vector.tensor_scalar(acc1[:], Xf, 1.0, sc2,
                                                ALU.is_equal, ALU.mult)
                    elif gp and k == 2:
                        nc.vector.tensor_scalar(accg[:], Xf, 2.0, sc2,
                                                ALU.is_equal, ALU.mult)
                    elif gp and k < 2 + gp:
                        tg = dqp.tile([128, GW], dt.bfloat16, tag="tg",
                                      bufs=4, name="tg")
                        nc.vector.tensor_scalar(tg[:], Xf, float(k), sc2,
                                                ALU.is_equal, ALU.mult)
                        nc.gpsimd.tensor_tensor(accg[:], accg[:], tg[:],
                                                ALU.add)
                    else:
                        tk = dqp.tile([128, GW], dt.bfloat16, tag="tk",
                                      bufs=1, name="tk")
                        nc.vector.tensor_scalar(tk[:], Xf, float(k), sc2,
                                                ALU.is_equal, ALU.mult)
                        acc = acc0 if (nterm % 2 == 0) else acc1
                        nterm += 1
                        nc.vector.tensor_tensor(acc[:], acc[:], tk[:], ALU.add)
                nc.vector.tensor_tensor(acc0[:], acc0[:], acc1[:], ALU.add)
                if gp:
                    nc.vector.tensor_tensor(acc0[:], acc0[:], accg[:], ALU.add)
                # blockwise absmax scale (host pre-expanded plane)
                wflat = wview.rearrange("p g f -> p (g f)")
                nc.vector.tensor_tensor(wflat[:], acc0[:],
                                        S[:].rearrange("p g f -> p (g f)"),
                                        ALU.mult)
                if out_dram is not None:
                    nc.sync.dma_start(
                        out_dram[128 * c0:128 * (c0 + G), f0:f0 + fw]
                        .rearrange("(g p) f -> p g f", p=128),
                        wview)

            def make_gu_tasks(si, fg0):
                nfg = cfg.slices[si]
                fw = nfg * 128
                wg, wu = wtiles[si]
                out = []
                for (c0, G) in cfg.slice_groups(nfg):
                    out.append(lambda c0=c0, G=G: emit_deq(
                        g_nib, g_amp, c0, G, fg0 * 128, fw,
                        wg[:, c0:c0 + G, :], cfg.gp_terms))
                    out.append(lambda c0=c0, G=G: emit_deq(
                        u_nib, u_amp, c0, G, fg0 * 128, fw,
                        wu[:, c0:c0 + G, :], cfg.gp_terms))
                return out

            def make_down_tasks():
                out = []
                npc = max(1, D // cfg.deq_w)    # col pieces per chunk
                pw = D // npc
                for c in range(NFG):
                    for h in range(npc):
                        def f(c=c, h=h):
                            wt = dqp.tile([128, 1, pw], dt.bfloat16,
                                          tag="wdt", bufs=1, name="wdt")
                            emit_deq(d_nib, d_amp, c, 1, h * pw, pw, wt[:],
                                     cfg.gp_terms_down, out_dram=wd_dram)
                        out.append(f)
                return out

            n_xh = 2 if cfg.DCH >= 8 else 1
            CH = cfg.DCH // n_xh      # chunks per x-half

            fg0s = np.cumsum([0] + cfg.slices).tolist()
            nsl = len(cfg.slices)

            open_wpool(0)
            tasks.extend(make_gu_tasks(0, fg0s[0]))
            pop_tasks(len(tasks))       # slice 0 dequant upfront
            if nsl > 1:
                open_wpool(1)
                tasks.extend(make_gu_tasks(1, fg0s[1]))
            down_added = nsl <= 1
            if down_added:
                tasks.extend(make_down_tasks())

            for si in range(nsl):
                fg0, nfg = fg0s[si], cfg.slices[si]
                fw = nfg * 128
                wg, wu = wtiles[si]
                quota = (len(tasks) + cfg.NT - 1) // cfg.NT if tasks else 0

                for t in range(cfg.NT):
                    tt = slice(TTW * t, TTW * (t + 1))
                    xth = []
                    for h in range(n_xh):
                        xh = xtp.tile([128, CH, TTW], dt.bfloat16, tag="xt",
                                      bufs=n_xh, name="xh")
                        nc.sync.dma_start(
                            xh[:], xT[128 * CH * h:128 * CH * (h + 1), tt]
                            .rearrange("(c p) t -> p c t", p=128))
                        xth.append(xh)

                    if si == 0:
                        pxag = psp.tile([R, TTW], dt.float32, tag="pxa",
                                        bufs=2, name="pxag")
                        pxau = psp.tile([R, TTW], dt.float32, tag="pxa",
                                        bufs=2, name="pxau")
                        for ci in range(cfg.DCH):
                            nc.tensor.matmul(
                                pxag[:], agu_sb[:, ci, 0:R],
                                xth[ci // CH][:, ci % CH, :],
                                start=(ci == 0), stop=(ci == cfg.DCH - 1))
                        for ci in range(cfg.DCH):
                            nc.tensor.matmul(
                                pxau[:], agu_sb[:, ci, R:2 * R],
                                xth[ci // CH][:, ci % CH, :],
                                start=(ci == 0), stop=(ci == cfg.DCH - 1))
                        nc.scalar.copy(xag_sb[:, tt], pxag[:])
                        nc.scalar.copy(xau_sb[:, tt], pxau[:])

                    x3b = p1p.tile([128, nfg, TTW], dt.bfloat16, tag="x3b",
                                   bufs=2, name="x3b")
                    for fg in range(nfg):
                        fa = slice(128 * (fg0 + fg), 128 * (fg0 + fg + 1))
                        fl = slice(128 * fg, 128 * (fg + 1))
                        pg = psp.tile([128, TTW], dt.float32, tag="pg", bufs=2,
                                      name="pg")
                        pu = psp.tile([128, TTW], dt.float32, tag="pu", bufs=2,
                                      name="pu")
                        for ci in range(cfg.DCH):
                            nc.tensor.matmul(pg[:], wg[:, ci, fl],
                                             xth[ci // CH][:, ci % CH, :],
                                             start=(ci == 0), stop=False)
                        nc.tensor.matmul(pg[:], bg_sb[:, fa], xag_sb[:, tt],
                                         start=False, stop=True)
                        for ci in range(cfg.DCH):
                            nc.tensor.matmul(pu[:], wu[:, ci, fl],
                                             xth[ci // CH][:, ci % CH, :],
                                             start=(ci == 0), stop=False)
                        nc.tensor.matmul(pu[:], bu_sb[:, fa], xau_sb[:, tt],
                                         start=False, stop=True)
                        pub = p1p.tile([128, TTW], dt.bfloat16, tag="pub",
                                       bufs=2, name="pub")
                        nc.scalar.copy(pub[:], pu[:])
                        if cfg.use_silu:
                            nc.scalar.activation(x3b[:, fg, :], pg[:],
                                                 AFT.Silu)
                            nc.gpsimd.tensor_tensor(x3b[:, fg, :],
                                                    x3b[:, fg, :], pub[:],
                                                    ALU.mult)
                        else:
                            sg = p1p.tile([128, TTW], dt.bfloat16, tag="sg",
                                          bufs=2, name="sg")
                            nc.scalar.activation(sg[:], pg[:], AFT.Sigmoid)
                            pgb = p1p.tile([128, TTW], dt.bfloat16, tag="pgb",
                                           bufs=2, name="pgb")
                            nc.scalar.copy(pgb[:], pg[:])
                            nc.gpsimd.tensor_tensor(sg[:], sg[:], pgb[:],
                                                    ALU.mult)
                            nc.gpsimd.tensor_tensor(x3b[:, fg, :], sg[:],
                                                    pub[:], ALU.mult)
                    nc.sync.dma_start(
                        x3_dram[128 * fg0:128 * fg0 + fw, tt]
                        .rearrange("(g p) t -> p g t", p=128),
                        x3b[:])
                    pop_tasks(quota)

                # queue what dequants next
                if si + 2 < nsl:
                    open_wpool(si + 2)
                    tasks.extend(make_gu_tasks(si + 2, fg0s[si + 2]))
                elif not down_added:
                    down_added = True
                    tasks.extend(make_down_tasks())

            pop_tasks(len(tasks))       # down-weight dequant tail

            # phase-2 prologue: x3a = Ad^T @ x3 (overlaps the dequant tail)
            for tg2 in range(cfg.NTG):
                tsl = slice(128 * tg2, 128 * (tg2 + 1))
                x3p = p1p.tile([128, NFG, 128], dt.bfloat16, tag="x3b",
                               bufs=2, name="x3p")
                nc.sync.dma_start(
                    x3p[:], x3_dram[:, tsl].rearrange("(c p) t -> p c t",
                                                      p=128))
                px3a = psp.tile([R, 128], dt.float32, tag="px3a", name="px3a")
                for ci in range(NFG):
                    nc.tensor.matmul(px3a[:], ad_sb[:, ci, :], x3p[:, ci, :],
                                     start=(ci == 0), stop=(ci == NFG - 1))
                nc.scalar.copy(x3a_sb[:, tsl], px3a[:])

        dqp_cm.__exit__(None, None, None)

        # ------------- phase 2 -------------
        with tc.tile_pool(name="p2", bufs=1) as p2p, \
             tc.tile_pool(name="wd", bufs=1) as wdp, \
             tc.tile_pool(name="ps2", bufs=1, space="PSUM") as ps2:
            bd_sb = p2p.tile([R, D], dt.bfloat16, tag="bd", name="bd_sb")
            nc.sync.dma_start(bd_sb[:], b_d[:])
            n_dj = cfg.DH // 512
            for dh in range(cfg.n_dh):
                dsl = slice(cfg.DH * dh, cfg.DH * (dh + 1))
                wd_sb = wdp.tile([128, NFG, cfg.DH], dt.bfloat16, tag="wd",
                                 bufs=min(2, cfg.n_dh), name="wd_sb")
                nc.sync.dma_start(
                    wd_sb[:], wd_dram[:, dsl].rearrange("(c p) d -> p c d",
                                                        p=128))
                for q in range(cfg.n_q):
                    j = (dh * cfg.n_q + q) % 2
                    for tgl in range(cfg.TQ // 128):
                        tg = (cfg.TQ // 128) * q + tgl
                        tsl = slice(128 * tg, 128 * (tg + 1))
                        x3g = p2p.tile([128, NFG, 128], dt.bfloat16, tag="x3g",
                                       bufs=3, name="x3g")
                        nc.sync.dma_start(
                            x3g[:], x3_dram[:, tsl]
                            .rearrange("(c p) t -> p c t", p=128))
                        pds = [ps2.tile([128, 512], dt.float32, tag="pd",
                                        bufs=8, name=f"pd{dj}")
                               for dj in range(n_dj)]
                        for ci in range(NFG):
                            for dj in range(n_dj):
                                nc.tensor.matmul(
                                    pds[dj][:], x3g[:, ci, :],
                                    wd_sb[:, ci, 512 * dj:512 * (dj + 1)],
                                    start=(ci == 0), stop=False)
                        for dj in range(n_dj):
                            nc.tensor.matmul(
                                pds[dj][:], x3a_sb[:, tsl],
                                bd_sb[:, cfg.DH * dh + 512 * dj:
                                      cfg.DH * dh + 512 * (dj + 1)],
                                start=False, stop=True)
                        yb = p2p.tile([128, cfg.DH], dt.bfloat16, tag="yb",
                                      bufs=2, name="yb")
                        for dj in range(n_dj):
                            nc.scalar.copy(yb[:, 512 * dj:512 * (dj + 1)],
                                           pds[dj][:])
                        nc.sync.dma_start(
                            rs_in[j][128 * tgl:128 * (tgl + 1), :], yb[:])
                    nc.gpsimd.collective_compute(
                        "ReduceScatter", ALU.add, replica_groups=rg,
                        ins=[rs_in[j][:, :].opt()],
                        outs=[rs_out[j][:, :].opt()],
                    )
                    # convert + emit this quarter's output rows on DVE (so the
                    # ACT stream never blocks on the collective)
                    for r0 in range(0, cfg.TQC, 128):
                        rw = min(128, cfg.TQC - r0)
                        rt = p2p.tile([128, cfg.DH], dt.bfloat16, tag="rt",
                                      bufs=2, name="rt")
                        nc.sync.dma_start(rt[0:rw, :],
                                          rs_out[j][r0:r0 + rw, :])
                        yf = p2p.tile([128, cfg.DH], dt.float32, tag="yf",
                                      bufs=2, name="yf")
                        nc.vector.tensor_scalar(yf[0:rw, :], rt[0:rw, :], 1.0,
                                                None, ALU.mult)
                        nc.sync.dma_start(
                            y_out[cfg.TQC * q + r0:cfg.TQC * q + r0 + rw, dsl],
                            yf[0:rw, :])

    nc.compile()
    return nc


# ----------------- host side -----------------

_CACHE = {}


def _get_graph(cfg: Cfg):
    key = (cfg.D, cfg.T, cfg.F, cfg.ncores, cfg.use_silu)
    if key not in _CACHE:
        _CACHE[key] = build_graph(cfg)
    return _CACHE[key]


def _prep_inputs(cfg: Cfg, inputs):
    """Shard + lay out the full inputs for each core (marshalling only:
    transpose, nibble unpack, dtype casts, padding)."""
    D, T, F, FP, FS, R = cfg.D, cfg.T, cfg.F, cfg.FP, cfg.FS, cfg.R
    blk = cfg.block

    x = np.asarray(inputs["x"])
    xT = np.ascontiguousarray(x.T).astype(BF16)

    def nib_split(packed, rows, cols):
        """packed int32 words (one byte each) -> u8 nibble values [rows, cols]."""
        b = (np.asarray(packed).astype(np.int64) & 0xFF).astype(np.uint8)
        b = b.reshape(rows, cols // 2)
        out = np.empty((rows, cols), np.uint8)
        out[:, 0::2] = b >> 4
        out[:, 1::2] = b & 0xF
        return out

    # gate/up: [F, D] -> pad rows to FP -> transpose -> [D, FP]; shard cols
    def prep_gu(packed, absmax):
        nib = nib_split(packed, F, D)
        nib = np.concatenate([nib, np.zeros((FP - F, D), np.uint8)], 0)
        nibT = np.ascontiguousarray(nib.T)              # [D, FP]
        am = np.asarray(absmax, np.float32).reshape(F, D // blk)
        am = np.concatenate([am, np.zeros((FP - F, D // blk), np.float32)], 0)
        amT = np.ascontiguousarray(am.T).astype(BF16)   # [D/blk, FP]
        return nibT, amT

    g_nibT, g_amT = prep_gu(inputs["w_gate_packed"], inputs["w_gate_absmax"])
    u_nibT, u_amT = prep_gu(inputs["w_up_packed"], inputs["w_up_absmax"])

    # down: [D, F] -> pad cols to FP -> transpose -> [FP, D]; shard rows
    d_nib = nib_split(inputs["w_down_packed"], D, F)
    d_nib = np.concatenate([d_nib, np.zeros((D, FP - F), np.uint8)], 1)
    d_nibT = np.ascontiguousarray(d_nib.T)              # [FP, D]
    d_am = np.asarray(inputs["w_down_absmax"], np.float32).reshape(D, F // blk)
    d_am = np.concatenate([d_am, np.zeros((D, (FP - F) // blk), np.float32)], 1)
    d_amT = np.ascontiguousarray(d_am.T).astype(BF16)   # [FP/blk, D]

    code_rep = np.broadcast_to(
        np.asarray(inputs["code"]).astype(np.float32)[None, :], (128, 16)
    ).copy()
    a_gu = np.concatenate(
        [np.asarray(inputs["w_gate_lora_a"]),
         np.asarray(inputs["w_up_lora_a"])], axis=1).astype(BF16)

    def pad_cols(m):
        return np.concatenate([m, np.zeros((m.shape[0], FP - F), m.dtype)], 1)

    b_g_full = pad_cols(np.asarray(inputs["w_gate_lora_b"], np.float32))
    b_u_full = pad_cols(np.asarray(inputs["w_up_lora_b"], np.float32))
    a_d_full = np.concatenate(
        [np.asarray(inputs["w_down_lora_a"], np.float32),
         np.zeros((FP - F, R), np.float32)], 0)
    b_d = np.asarray(inputs["w_down_lora_b"]).astype(BF16)

    g_amP = np.repeat(g_amT, blk, axis=0)     # [D, FP]
    u_amP = np.repeat(u_amT, blk, axis=0)
    d_amP = np.repeat(d_amT, blk, axis=0)     # [FP, D]

    in_maps = []
    for i in range(cfg.ncores):
        fsl = slice(FS * i, FS * (i + 1))
        in_maps.append({
            "xT": xT,
            "g_nib": np.ascontiguousarray(g_nibT[:, fsl]),
            "u_nib": np.ascontiguousarray(u_nibT[:, fsl]),
            "d_nib": np.ascontiguousarray(d_nibT[fsl]),
            "g_amp": np.ascontiguousarray(g_amP[:, fsl]),
            "u_amp": np.ascontiguousarray(u_amP[:, fsl]),
            "d_amp": np.ascontiguousarray(d_amP[fsl]),
            "code_rep": code_rep,
            "a_gu": a_gu,
            "b_g": np.ascontiguousarray(b_g_full[:, fsl]).astype(BF16),
            "b_u": np.ascontiguousarray(b_u_full[:, fsl]).astype(BF16),
            "a_d": np.ascontiguousarray(a_d_full[fsl]).astype(BF16),
            "b_d": b_d,
        })
    return in_maps


def _gather(cfg: Cfg, results):
    """Reassemble full [T, D] output from per-core quarter-row blocks."""
    y = np.empty((cfg.T, cfg.D), np.float32)
    for i in range(cfg.ncores):
        yi = results[i]["y_out"]
        for q in range(cfg.n_q):
            r0 = cfg.TQ * q + cfg.TQC * i
            y[r0:r0 + cfg.TQC] = yi[cfg.TQC * q:cfg.TQC * (q + 1)]
    return y


def run(cfg: Cfg, inputs, trace=False, **kwargs):
    nc = _get_graph(cfg)
    in_maps = _prep_inputs(cfg, inputs)
    res = run_bass_kernel_spmd(
        nc, in_maps, core_ids=list(range(cfg.ncores)), trace=trace, **kwargs
    )
    y = _gather(cfg, res.results)
    return y, res


def kernel(**inputs) -> np.ndarray:
    cfg = Cfg()
    y, _ = run(cfg, inputs)
    return y.astype(np.float32)


# revision 28
# speedup vs baseline: 1.0359x; 1.0359x over previous
"""Trainium2 Bass kernel for nn_MixedGatedMLP (4-bit quantized gated MLP + LoRA).

Strategy v2: tensor-parallel over d_ff across 8 NeuronCores (F padded
11008->11264, FS=1408 rows/core).  Host ships unpacked nibble planes (u8) and
compact per-64-block absmax rows; the device does the codebook lookup
(16x is_equal/mult tensor_scalar at 4x DVE rate + an add tree partially
offloaded to GPSIMD) and blockwise scaling (scale plane expanded on PE via a
2->128 one-hot matmul, multiplied straight out of PSUM).

Phase 1 streams x in token tiles over ramped f-slices of the gate/up weights;
dequant of slice s+1 overlaps the matmuls of slice s (issue-order zip).
SiLU runs on ACT, the gating multiply on GPSIMD, and x3 goes to DRAM.
The down weights dequant during the last slices' matmuls and round-trip
through DRAM.  Phase 2 recomputes y3 = x3 @ wd + lora in d-halves, with a
token-quartered bf16 ReduceScatter overlapping the next quarter's matmuls.
Core i returns tokens {TQ*q + TQC*i ..} for each quarter q; the host
reassembles.
"""

import sys

for _p in ("/opt/trn_rl_repo", "/root/.axon_site/_ro/trn_rl_repo"):
    if _p not in sys.path:
        sys.path.append(_p)

from contextlib import ExitStack

import numpy as np
import ml_dtypes

import concourse.bass as bass
import concourse.mybir as mybir
import concourse.tile as tile
from concourse import bacc
from concourse.bass_utils import run_bass_kernel_spmd

BF16 = ml_dtypes.bfloat16
ALU = mybir.AluOpType
AFT = mybir.ActivationFunctionType


class Cfg:
    def __init__(self, D=4096, T=4096, F=11008, R=16, block=64, ncores=8,
                 use_silu=True):
        self.D = D
        self.T = T
        self.F = F
        self.R = R
        self.block = block
        self.ncores = ncores
        self.use_silu = use_silu

        unit = 128 * ncores
        self.FP = ((F + unit - 1) // unit) * unit   # padded d_ff
        self.FS = self.FP // ncores                 # per-core f rows
        self.NFG = self.FS // 128                   # 128-row f groups
        self.DCH = D // 128                         # 128-row d chunks
        self.NAB = D // block                       # absmax rows (gate/up)
        self.ABC = 128 // block                     # absmax blocks per chunk

        # token tiling
        self.TTW = min(512, T)                      # phase-1 token tile width
        self.NT = T // self.TTW
        self.NTG = T // 128                         # phase-2 token groups

        # phase-1 f slices in fgroup units: ramped for a small first bubble
        if self.NFG == 11:
            self.slices = [1, 2, 3, 2, 3]
        else:
            self.slices = [1] * self.NFG

        self.gp_terms = 6        # codebook terms handled by gpsimd (g/u)
        self.gp_terms_down = 6   # ... for the down weights (gpsimd idle then)
        self.deq_w = 2048        # target dequant op width (free elems)

        # phase 2
        self.n_dh = 2 if D >= 2048 else 1           # d halves
        self.DH = D // self.n_dh
        self.n_q = 4 if T >= 4096 else 2            # token quarters (RS chunks)
        self.TQ = T // self.n_q
        self.TQC = self.TQ // ncores                # rows per core per quarter
        self.TS = T // ncores

    def slice_groups(self, nfg):
        """Dequant chunk-groups (c0, G) for a slice of nfg fgroups."""
        fw = nfg * 128
        G = max(1, min(self.deq_w // fw, 8, self.DCH))
        while self.DCH % G:
            G -= 1
        return [(c0, G) for c0 in range(0, self.DCH, G)]


def build_graph(cfg: Cfg):
    nc = bacc.Bacc(None, num_devices=cfg.ncores)
    dt = mybir.dt
    D, T, FS, R, NFG = cfg.D, cfg.T, cfg.FS, cfg.R, cfg.NFG
    TTW = cfg.TTW
    rg = [list(range(cfg.ncores))]

    # ---- external inputs (per-core) ----
    xT = nc.dram_tensor("xT", [D, T], dt.bfloat16, kind="ExternalInput")
    g_nib = nc.dram_tensor("g_nib", [D, FS], dt.bfloat16, kind="ExternalInput")
    u_nib = nc.dram_tensor("u_nib", [D, FS], dt.bfloat16, kind="ExternalInput")
    d_nib = nc.dram_tensor("d_nib", [FS, D], dt.bfloat16, kind="ExternalInput")
    g_amp = nc.dram_tensor("g_amp", [D, FS], dt.bfloat16, kind="ExternalInput")
    u_amp = nc.dram_tensor("u_amp", [D, FS], dt.bfloat16, kind="ExternalInput")
    d_amp = nc.dram_tensor("d_amp", [FS, D], dt.bfloat16, kind="ExternalInput")
    code_rep = nc.dram_tensor("code_rep", [128, 16], dt.float32, kind="ExternalInput")
    a_gu = nc.dram_tensor("a_gu", [D, 2 * R], dt.bfloat16, kind="ExternalInput")
    b_g = nc.dram_tensor("b_g", [R, FS], dt.bfloat16, kind="ExternalInput")
    b_u = nc.dram_tensor("b_u", [R, FS], dt.bfloat16, kind="ExternalInput")
    a_d = nc.dram_tensor("a_d", [FS, R], dt.bfloat16, kind="ExternalInput")
    b_d = nc.dram_tensor("b_d", [R, D], dt.bfloat16, kind="ExternalInput")

    y_out = nc.dram_tensor("y_out", [cfg.TS, D], dt.float32, kind="ExternalOutput")

    # ---- internal DRAM ----
    x3_dram = nc.dram_tensor("x3_dram", [FS, T], dt.bfloat16, kind="Internal")
    wd_dram = nc.dram_tensor("wd_dram", [FS, D], dt.bfloat16, kind="Internal")
    rs_in = [
        nc.dram_tensor(f"rs_in{i}", [cfg.TQ, cfg.DH], dt.bfloat16, kind="Internal")
        for i in range(2)
    ]
    rs_out = [
        nc.dram_tensor(f"rs_out{i}", [cfg.TQC, cfg.DH], dt.bfloat16,
                       kind="Internal")
        for i in range(2)
    ]

    with tile.TileContext(nc) as tc, ExitStack() as ctx:
        # ---------------- constants ----------------
        cpool = ctx.enter_context(tc.tile_pool(name="const", bufs=1))
        code_sb = cpool.tile([128, 16], dt.float32)
        nc.sync.dma_start(code_sb[:], code_rep[:])
        agu_sb = cpool.tile([128, cfg.DCH, 2 * R], dt.bfloat16)
        nc.sync.dma_start(agu_sb[:], a_gu.rearrange("(c p) r -> p c r", p=128))
        bg_sb = cpool.tile([R, FS], dt.bfloat16)
        nc.sync.dma_start(bg_sb[:], b_g[:])
        bu_sb = cpool.tile([R, FS], dt.bfloat16)
        nc.sync.dma_start(bu_sb[:], b_u[:])
        ad_sb = cpool.tile([128, NFG, R], dt.bfloat16)
        nc.sync.dma_start(ad_sb[:], a_d.rearrange("(c p) r -> p c r", p=128))
        xag_sb = cpool.tile([R, T], dt.bfloat16)
        xau_sb = cpool.tile([R, T], dt.bfloat16)
        x3a_sb = cpool.tile([R, T], dt.bfloat16)

        dqp_cm = tc.tile_pool(name="dq", bufs=1)
        dqp = dqp_cm.__enter__()

        # ------------- phase 1 + dequant -------------
        wtiles = {}          # si -> (wg, wu); even/odd parity tags share bufs
        tasks = []

        def pop_tasks(n):
            for _ in range(n):
                if tasks:
                    tasks.pop(0)()

        with tc.tile_pool(name="w", bufs=1) as wp, \
             tc.tile_pool(name="xt", bufs=1) as xtp, \
             tc.tile_pool(name="p1", bufs=1) as p1p, \
             tc.tile_pool(name="ps1", bufs=1, space="PSUM") as psp:

            def open_wpool(si):
                if si in wtiles:
                    return
                fw = cfg.slices[si] * 128
                par = "eo"[si % 2]
                wg = wp.tile([128, cfg.DCH, fw], dt.bfloat16, tag=f"wg{par}",
                             name=f"wg{si}")
                wu = wp.tile([128, cfg.DCH, fw], dt.bfloat16, tag=f"wu{par}",
                             name=f"wu{si}")
                wtiles[si] = (wg, wu)

            def emit_deq(nib, amp, c0, G, f0, fw, wview, gp, out_dram=None):
                """Dequant chunk rows [128*c0, 128*(c0+G)) x cols [f0, f0+fw)
                into bf16 weights at `wview` ([128, G, fw] AP); optionally DMA
                to out_dram."""
                GW = G * fw
                X = dqp.tile([128, G, fw], dt.bfloat16, tag="x", bufs=2,
                             name="x")
                nc.sync.dma_start(
                    X[:], nib[128 * c0:128 * (c0 + G), f0:f0 + fw]
                    .rearrange("(g p) f -> p g f", p=128))
                S = dqp.tile([128, G, fw], dt.bfloat16, tag="am", bufs=1,
                             name="S")
                nc.sync.dma_start(
                    S[:], amp[128 * c0:128 * (c0 + G), f0:f0 + fw]
                    .rearrange("(g p) f -> p g f", p=128))
                Xf = X[:].rearrange("p g f -> p (g f)")
                acc0 = dqp.tile([128, GW], dt.bfloat16, tag="a0", name="a0")
                acc1 = dqp.tile([128, GW], dt.bfloat16, tag="a1", name="a1")
                if GW < 1024:
                    gp = 0
                accg = None
                if gp:
                    accg = dqp.tile([128, GW], dt.bfloat16, tag="ag",
                                    name="ag")
                nterm = 0
                for k in range(16):
                    sc2 = code_sb[:, k:k + 1]
                    if k == 0:
                        nc.vector.tensor_scalar(acc0[:], Xf, 0.0, sc2,
                                                ALU.is_equal, ALU.mult)
                    elif k == 1:
                        nc.vector.tensor_scalar(acc1[:], Xf, 1.0, sc2,
                                                ALU.is_equal, ALU.mult)
                    elif gp and k == 2:
                        nc.vector.tensor_scalar(accg[:], Xf, 2.0, sc2,
                                                ALU.is_equal, ALU.mult)
                    elif gp and k < 2 + gp:
                        tg = dqp.tile([128, GW], dt.bfloat16, tag="tg",
                                      bufs=6, name="tg")
                        nc.vector.tensor_scalar(tg[:], Xf, float(k), sc2,
                                                ALU.is_equal, ALU.mult)
                        nc.gpsimd.tensor_tensor(accg[:], accg[:], tg[:],
                                                ALU.add)
                    else:
                        tk = dqp.tile([128, GW], dt.bfloat16, tag="tk",
                                      bufs=1, name="tk")
                        nc.vector.tensor_scalar(tk[:], Xf, float(k), sc2,
                                                ALU.is_equal, ALU.mult)
                        acc = acc0 if (nterm % 2 == 0) else acc1
                        nterm += 1
                        nc.vector.tensor_tensor(acc[:], acc[:], tk[:], ALU.add)
                nc.vector.tensor_tensor(acc0[:], acc0[:], acc1[:], ALU.add)
                if gp:
                    nc.vector.tensor_tensor(acc0[:], acc0[:], accg[:], ALU.add)
                # blockwise absmax scale (host pre-expanded plane)
                wflat = wview.rearrange("p g f -> p (g f)")
                nc.vector.tensor_tensor(wflat[:], acc0[:],
                                        S[:].rearrange("p g f -> p (g f)"),
                                        ALU.mult)
                if out_dram is not None:
                    nc.sync.dma_start(
                        out_dram[128 * c0:128 * (c0 + G), f0:f0 + fw]
                        .rearrange("(g p) f -> p g f", p=128),
                        wview)

            def make_gu_tasks(si, fg0):
                nfg = cfg.slices[si]
                fw = nfg * 128
                wg, wu = wtiles[si]
                out = []
                for (c0, G) in cfg.slice_groups(nfg):
                    out.append(lambda c0=c0, G=G: emit_deq(
                        g_nib, g_amp, c0, G, fg0 * 128, fw,
                        wg[:, c0:c0 + G, :], cfg.gp_terms))
                    out.append(lambda c0=c0, G=G: emit_deq(
                        u_nib, u_amp, c0, G, fg0 * 128, fw,
                        wu[:, c0:c0 + G, :], cfg.gp_terms))
                return out

            def make_down_tasks():
                out = []
                npc = max(1, D // cfg.deq_w)    # col pieces per chunk
                pw = D // npc
                for c in range(NFG):
                    for h in range(npc):
                        def f(c=c, h=h):
                            wt = dqp.tile([128, 1, pw], dt.bfloat16,
                                          tag="wdt", bufs=1, name="wdt")
                            emit_deq(d_nib, d_amp, c, 1, h * pw, pw, wt[:],
                                     cfg.gp_terms_down, out_dram=wd_dram)
                        out.append(f)
                return out

            n_xh = 2 if cfg.DCH >= 8 else 1
            CH = cfg.DCH // n_xh      # chunks per x-half

            fg0s = np.cumsum([0] + cfg.slices).tolist()
            nsl = len(cfg.slices)

            open_wpool(0)
            tasks.extend(make_gu_tasks(0, fg0s[0]))
            pop_tasks(len(tasks))       # slice 0 dequant upfront
            if nsl > 1:
                open_wpool(1)
                tasks.extend(make_gu_tasks(1, fg0s[1]))
            down_added = nsl <= 1
            if down_added:
                tasks.extend(make_down_tasks())

            for si in range(nsl):
                fg0, nfg = fg0s[si], cfg.slices[si]
                fw = nfg * 128
                wg, wu = wtiles[si]
                quota = (len(tasks) + cfg.NT - 1) // cfg.NT if tasks else 0

                for t in range(cfg.NT):
                    tt = slice(TTW * t, TTW * (t + 1))
                    xth = []
                    for h in range(n_xh):
                        xh = xtp.tile([128, CH, TTW], dt.bfloat16, tag="xt",
                                      bufs=n_xh, name="xh")
                        nc.sync.dma_start(
                            xh[:], xT[128 * CH * h:128 * CH * (h + 1), tt]
                            .rearrange("(c p) t -> p c t", p=128))
                        xth.append(xh)

                    if si == 0:
                        pxag = psp.tile([R, TTW], dt.float32, tag="pxa",
                                        bufs=2, name="pxag")
                        pxau = psp.tile([R, TTW], dt.float32, tag="pxa",
                                        bufs=2, name="pxau")
                        for ci in range(cfg.DCH):
                            nc.tensor.matmul(
                                pxag[:], agu_sb[:, ci, 0:R],
                                xth[ci // CH][:, ci % CH, :],
                                start=(ci == 0), stop=(ci == cfg.DCH - 1))
                        for ci in range(cfg.DCH):
                            nc.tensor.matmul(
                                pxau[:], agu_sb[:, ci, R:2 * R],
                                xth[ci // CH][:, ci % CH, :],
                                start=(ci == 0), stop=(ci == cfg.DCH - 1))
                        nc.scalar.copy(xag_sb[:, tt], pxag[:])
                        nc.scalar.copy(xau_sb[:, tt], pxau[:])

                    x3b = p1p.tile([128, nfg, TTW], dt.bfloat16, tag="x3b",
                                   bufs=2, name="x3b")
                    for fg in range(nfg):
                        fa = slice(128 * (fg0 + fg), 128 * (fg0 + fg + 1))
                        fl = slice(128 * fg, 128 * (fg + 1))
                        pg = psp.tile([128, TTW], dt.float32, tag="pg", bufs=2,
                                      name="pg")
                        pu = psp.tile([128, TTW], dt.float32, tag="pu", bufs=2,
                                      name="pu")
                        for ci in range(cfg.DCH):
                            nc.tensor.matmul(pg[:], wg[:, ci, fl],
                                             xth[ci // CH][:, ci % CH, :],
                                             start=(ci == 0), stop=False)
                        nc.tensor.matmul(pg[:], bg_sb[:, fa], xag_sb[:, tt],
                                         start=False, stop=True)
                        for ci in range(cfg.DCH):
                            nc.tensor.matmul(pu[:], wu[:, ci, fl],
                                             xth[ci // CH][:, ci % CH, :],
                                             start=(ci == 0), stop=False)
                        nc.tensor.matmul(pu[:], bu_sb[:, fa], xau_sb[:, tt],
                                         start=False, stop=True)
                        pub = p1p.tile([128, TTW], dt.bfloat16, tag="pub",
                                       bufs=1, name="pub")
                        nc.scalar.copy(pub[:], pu[:])
                        if cfg.use_silu:
                            nc.scalar.activation(x3b[:, fg, :], pg[:],
                                                 AFT.Silu)
                            nc.gpsimd.tensor_tensor(x3b[:, fg, :],
                                                    x3b[:, fg, :], pub[:],
                                                    ALU.mult)
                        else:
                            sg = p1p.tile([128, TTW], dt.bfloat16, tag="sg",
                                          bufs=2, name="sg")
                            nc.scalar.activation(sg[:], pg[:], AFT.Sigmoid)
                            pgb = p1p.tile([128, TTW], dt.bfloat16, tag="pgb",
                                           bufs=2, name="pgb")
                            nc.scalar.copy(pgb[:], pg[:])
                            nc.gpsimd.tensor_tensor(sg[:], sg[:], pgb[:],
                                                    ALU.mult)
                            nc.gpsimd.tensor_tensor(x3b[:, fg, :], sg[:],
                                                    pub[:], ALU.mult)
                    nc.sync.dma_start(
                        x3_dram[128 * fg0:128 * fg0 + fw, tt]
                        .rearrange("(g p) t -> p g t", p=128),
                        x3b[:])
                    pop_tasks(quota)

                # queue what dequants next
                if si + 2 < nsl:
                    open_wpool(si + 2)
                    tasks.extend(make_gu_tasks(si + 2, fg0s[si + 2]))
                elif not down_added:
                    down_added = True
                    tasks.extend(make_down_tasks())

            pop_tasks(len(tasks))       # down-weight dequant tail

            # phase-2 prologue: x3a = Ad^T @ x3 (overlaps the dequant tail)
            for tg2 in range(cfg.NTG):
                tsl = slice(128 * tg2, 128 * (tg2 + 1))
                x3p = p1p.tile([128, NFG, 128], dt.bfloat16, tag="x3b",
                               bufs=2, name="x3p")
                nc.sync.dma_start(
                    x3p[:], x3_dram[:, tsl].rearrange("(c p) t -> p c t",
                                                      p=128))
                px3a = psp.tile([R, 128], dt.float32, tag="px3a", name="px3a")
                for ci in range(NFG):
                    nc.tensor.matmul(px3a[:], ad_sb[:, ci, :], x3p[:, ci, :],
                                     start=(ci == 0), stop=(ci == NFG - 1))
                nc.scalar.copy(x3a_sb[:, tsl], px3a[:])

        dqp_cm.__exit__(None, None, None)

        # ------------- phase 2 -------------
        with tc.tile_pool(name="p2", bufs=1) as p2p, \
             tc.tile_pool(name="wd", bufs=1) as wdp, \
             tc.tile_pool(name="ps2", bufs=1, space="PSUM") as ps2:
            bd_sb = p2p.tile([R, D], dt.bfloat16, tag="bd", name="bd_sb")
            nc.sync.dma_start(bd_sb[:], b_d[:])
            n_dj = cfg.DH // 512
            for dh in range(cfg.n_dh):
                dsl = slice(cfg.DH * dh, cfg.DH * (dh + 1))
                wd_sb = wdp.tile([128, NFG, cfg.DH], dt.bfloat16, tag="wd",
                                 bufs=min(2, cfg.n_dh), name="wd_sb")
                nc.sync.dma_start(
                    wd_sb[:], wd_dram[:, dsl].rearrange("(c p) d -> p c d",
                                                        p=128))
                for q in range(cfg.n_q):
                    j = (dh * cfg.n_q + q) % 2
                    for tgl in range(cfg.TQ // 128):
                        tg = (cfg.TQ // 128) * q + tgl
                        tsl = slice(128 * tg, 128 * (tg + 1))
                        x3g = p2p.tile([128, NFG, 128], dt.bfloat16, tag="x3g",
                                       bufs=3, name="x3g")
                        nc.sync.dma_start(
                            x3g[:], x3_dram[:, tsl]
                            .rearrange("(c p) t -> p c t", p=128))
                        pds = [ps2.tile([128, 512], dt.float32, tag="pd",
                                        bufs=8, name=f"pd{dj}")
                               for dj in range(n_dj)]
                        for ci in range(NFG):
                            for dj in range(n_dj):
                                nc.tensor.matmul(
                                    pds[dj][:], x3g[:, ci, :],
                                    wd_sb[:, ci, 512 * dj:512 * (dj + 1)],
                                    start=(ci == 0), stop=False)
                        for dj in range(n_dj):
                            nc.tensor.matmul(
                                pds[dj][:], x3a_sb[:, tsl],
                                bd_sb[:, cfg.DH * dh + 512 * dj:
                                      cfg.DH * dh + 512 * (dj + 1)],
                                start=False, stop=True)
                        yb = p2p.tile([128, cfg.DH], dt.bfloat16, tag="yb",
                                      bufs=2, name="yb")
                        for dj in range(n_dj):
                            nc.scalar.copy(yb[:, 512 * dj:512 * (dj + 1)],
                                           pds[dj][:])
                        nc.sync.dma_start(
                            rs_in[j][128 * tgl:128 * (tgl + 1), :], yb[:])
                    nc.gpsimd.collective_compute(
                        "ReduceScatter", ALU.add, replica_groups=rg,
                        ins=[rs_in[j][:, :].opt()],
                        outs=[rs_out[j][:, :].opt()],
                    )
                    # convert + emit this quarter's output rows on DVE (so the
                    # ACT stream never blocks on the collective)
                    for r0 in range(0, cfg.TQC, 128):
                        rw = min(128, cfg.TQC - r0)
                        rt = p2p.tile([128, cfg.DH], dt.bfloat16, tag="rt",
                                      bufs=2, name="rt")
                        nc.sync.dma_start(rt[0:rw, :],
                                          rs_out[j][r0:r0 + rw, :])
                        yf = p2p.tile([128, cfg.DH], dt.float32, tag="yf",
                                      bufs=2, name="yf")
                        nc.vector.tensor_scalar(yf[0:rw, :], rt[0:rw, :], 1.0,
                                                None, ALU.mult)
                        nc.sync.dma_start(
                            y_out[cfg.TQC * q + r0:cfg.TQC * q + r0 + rw, dsl],
                            yf[0:rw, :])

    nc.compile()
    return nc


# ----------------- host side -----------------

_CACHE = {}


def _get_graph(cfg: Cfg):
    key = (cfg.D, cfg.T, cfg.F, cfg.ncores, cfg.use_silu)
    if key not in _CACHE:
        _CACHE[key] = build_graph(cfg)
    return _CACHE[key]


def _prep_inputs(cfg: Cfg, inputs):
    """Shard + lay out the full inputs for each core (marshalling only:
    transpose, nibble unpack, dtype casts, padding)."""
    D, T, F, FP, FS, R = cfg.D, cfg.T, cfg.F, cfg.FP, cfg.FS, cfg.R
    blk = cfg.block

    x = np.asarray(inputs["x"])
    xT = np.ascontiguousarray(x.T).astype(BF16)

    def nib_split(packed, rows, cols):
        """packed int32 words (one byte each) -> u8 nibble values [rows, cols]."""
        b = (np.asarray(packed).astype(np.int64) & 0xFF).astype(np.uint8)
        b = b.reshape(rows, cols // 2)
        out = np.empty((rows, cols), np.uint8)
        out[:, 0::2] = b >> 4
        out[:, 1::2] = b & 0xF
        return out

    # gate/up: [F, D] -> pad rows to FP -> transpose -> [D, FP]; shard cols
    def prep_gu(packed, absmax):
        nib = nib_split(packed, F, D)
        nib = np.concatenate([nib, np.zeros((FP - F, D), np.uint8)], 0)
        nibT = np.ascontiguousarray(nib.T).astype(BF16)  # [D, FP]
        am = np.asarray(absmax, np.float32).reshape(F, D // blk)
        am = np.concatenate([am, np.zeros((FP - F, D // blk), np.float32)], 0)
        amT = np.ascontiguousarray(am.T).astype(BF16)   # [D/blk, FP]
        return nibT, amT

    g_nibT, g_amT = prep_gu(inputs["w_gate_packed"], inputs["w_gate_absmax"])
    u_nibT, u_amT = prep_gu(inputs["w_up_packed"], inputs["w_up_absmax"])

    # down: [D, F] -> pad cols to FP -> transpose -> [FP, D]; shard rows
    d_nib = nib_split(inputs["w_down_packed"], D, F)
    d_nib = np.concatenate([d_nib, np.zeros((D, FP - F), np.uint8)], 1)
    d_nibT = np.ascontiguousarray(d_nib.T).astype(BF16)  # [FP, D]
    d_am = np.asarray(inputs["w_down_absmax"], np.float32).reshape(D, F // blk)
    d_am = np.concatenate([d_am, np.zeros((D, (FP - F) // blk), np.float32)], 1)
    d_amT = np.ascontiguousarray(d_am.T).astype(BF16)   # [FP/blk, D]

    code_rep = np.broadcast_to(
        np.asarray(inputs["code"]).astype(np.float32)[None, :], (128, 16)
    ).copy()
    a_gu = np.concatenate(
        [np.asarray(inputs["w_gate_lora_a"]),
         np.asarray(inputs["w_up_lora_a"])], axis=1).astype(BF16)

    def pad_cols(m):
        return np.concatenate([m, np.zeros((m.shape[0], FP - F), m.dtype)], 1)

    b_g_full = pad_cols(np.asarray(inputs["w_gate_lora_b"], np.float32))
    b_u_full = pad_cols(np.asarray(inputs["w_up_lora_b"], np.float32))
    a_d_full = np.concatenate(
        [np.asarray(inputs["w_down_lora_a"], np.float32),
         np.zeros((FP - F, R), np.float32)], 0)
    b_d = np.asarray(inputs["w_down_lora_b"]).astype(BF16)

    g_amP = np.repeat(g_amT, blk, axis=0)     # [D, FP]
    u_amP = np.repeat(u_amT, blk, axis=0)
    d_amP = np.repeat(d_amT, blk, axis=0)     # [FP, D]

    in_maps = []
    for i in range(cfg.ncores):
        fsl = slice(FS * i, FS * (i + 1))
        in_maps.append({
            "xT": xT,
            "g_nib": np.ascontiguousarray(g_nibT[:, fsl]),
            "u_nib": np.ascontiguousarray(u_nibT[:, fsl]),
            "d_nib": np.ascontiguousarray(d_nibT[fsl]),
            "g_amp": np.ascontiguousarray(g_amP[:, fsl]),
            "u_amp": np.ascontiguousarray(u_amP[:, fsl]),
            "d_amp": np.ascontiguousarray(d_amP[fsl]),
            "code_rep": code_rep,
            "a_gu": a_gu,
            "b_g": np.ascontiguousarray(b_g_full[:, fsl]).astype(BF16),
            "b_u": np.ascontiguousarray(b_u_full[:, fsl]).astype(BF16),
            "a_d": np.ascontiguousarray(a_d_full[fsl]).astype(BF16),
            "b_d": b_d,
        })
    return in_maps


def _gather(cfg: Cfg, results):
    """Reassemble full [T, D] output from per-core quarter-row blocks."""
    y = np.empty((cfg.T, cfg.D), np.float32)
    for i in range(cfg.ncores):
        yi = results[i]["y_out"]
        for q in range(cfg.n_q):
            r0 = cfg.TQ * q + cfg.TQC * i
            y[r0:r0 + cfg.TQC] = yi[cfg.TQC * q:cfg.TQC * (q + 1)]
    return y


def run(cfg: Cfg, inputs, trace=False, **kwargs):
    nc = _get_graph(cfg)
    in_maps = _prep_inputs(cfg, inputs)
    res = run_bass_kernel_spmd(
        nc, in_maps, core_ids=list(range(cfg.ncores)), trace=trace, **kwargs
    )
    y = _gather(cfg, res.results)
    return y, res


def kernel(**inputs) -> np.ndarray:
    cfg = Cfg()
    y, _ = run(cfg, inputs)
    return y.astype(np.float32)
